# revision 4
# baseline (speedup 1.0000x reference)
"""NodeSetAttention TRN2 Bass kernel, v2 (fully-transposed dataflow).

Shapes: B=4, S=2048, D=256, H=8, HD=32, E=65536. 8 cores: core c ->
(batch b=c//2, query half qh=c%2); queries are the first 1024 token
columns after rotation, keys are all 2048.

Design (cost model = instruction_cost_v2 / TimelineSim):
- everything stays transposed [feature, token]; LN mean/var come from
  PE column-sum matmuls (free size 1 each) finished in [128,16] column
  space, then transposed back to rows for broadcast/augmentation.
- LN affine folds into weights (g) and biases; the -mu*rho correction
  rides the projections as a K=1 augmentation pass against the u=mu*rho
  row; rho scales tokens via one stt on xT (xs = xT * rho_bcast).
- QK runs fp8e4m3 DoubleRow (0.5 cyc/row): q/k tiles [128, grp, slot, t]
  with slot1 zeroed; head h uses partitions 32*(h%4)..+32 of group h//4.
- exp flavors per kt over tiles [128 keys, 1024 queries]:
  * a2 (kt < A2_END): ACT exp -> f16; GPSIMD tensor_tensor mask-mul.
  * a (kt < A_END): ACT exp into head-pair tile; DVE stt 4x mask-mul
    with stride-0 expa broadcast over the pair.
  * b: DVE fused Schraudolph: i16 = psum*ASCH + C, bitcast f16.
    C = ASCH*(clip(bias)+SHIFT)+BSCH on edges else -60000 (saturates to
    -0.0); masked-in args verified inside (-10.35, 11.83).
- PV fp16 with a ones column for the denominator; psum groups of 4 qt
  per drain; normalize via reciprocal + stt with stride-0 broadcast.
- post-attention (O+LN2+MLP) per 512-token chunk; chunk 0 overlaps the
  last head's second PV group.
"""

import math
from contextlib import ExitStack

import numpy as np

import concourse.bass as bass
import concourse.bacc as bacc
import concourse.mybir as mybir
import concourse.tile as tile
from concourse.bass_utils import run_bass_kernel_spmd
from concourse.masks import make_identity

B, S, D, H, E = 4, 2048, 256, 8, 65536
HD = D // H
QH = S // 2
NCORES = 8
DT = S // 128
F32 = mybir.dt.float32
F16 = mybir.dt.float16
F8 = mybir.dt.float8e4
I16 = mybir.dt.int16
AF = mybir.ActivationFunctionType
OP = mybir.AluOpType
PM = mybir.MatmulPerfMode

SHIFT = 1.0
ASCH = float(2**10 / math.log(2.0))
BSCH = 15301.0
SCALE = 1.0 / math.sqrt(HD)

# per-kt exp flavor: a2 = ACT exp + GPSIMD mul, a = ACT exp + DVE mul,
# b = DVE fused Schraudolph, c = PE identity-add of log-mask + ACT exp.
# Interleaved so all four engines run concurrently.
KT_FLAVOR = ["b", "a2", "c", "b", "a2", "a", "b", "a2", "c", "a", "b",
             "a2", "a", "b", "a2", "b"]
CKTS = [kt for kt, f in enumerate(KT_FLAVOR) if f == "c"]
C_IDX = {kt: i for i, kt in enumerate(CKTS)}
KC = len(CKTS)
NC_F = KC
AKTS = [kt for kt, f in enumerate(KT_FLAVOR) if f in ("a", "a2")]

BKTS = [kt for kt, f in enumerate(KT_FLAVOR) if f == "b"]
A_IDX = {kt: i for i, kt in enumerate(AKTS)}
B_IDX = {kt: i for i, kt in enumerate(BKTS)}
KA = len(AKTS)
KB = len(BKTS)
NA2 = sum(1 for f in KT_FLAVOR if f == "a2")
NA = sum(1 for f in KT_FLAVOR if f == "a")
NB = KB


I32 = mybir.dt.int32


def _rsqrt_dve(nc, pool, var, out_f16, n, magic_sb):
    """out = 1/sqrt(var + eps) entirely on DVE: Quake bit trick + 1 Newton
    step (rel err ~2e-3, fine vs the 2e-2 gate). All ops tiny [128, n]."""
    ve = pool.tile([128, n], F32, tag=f"rs_ve{n}", name="ve")
    nc.vector.tensor_scalar(
        out=ve, in0=var, scalar1=1e-5, scalar2=None, op0=OP.add
    )
    sh = pool.tile([128, n], I32, tag=f"rs_sh{n}", name="sh")
    nc.vector.tensor_scalar(
        out=sh, in0=ve.bitcast(I32), scalar1=1, scalar2=None,
        op0=OP.logical_shift_right,
    )
    yi = pool.tile([128, n], I32, tag=f"rs_yi{n}", name="yi")
    nc.vector.tensor_tensor(
        out=yi, in0=magic_sb[:, 0:n].bitcast(I32), in1=sh, op=OP.subtract
    )
    y0 = yi.bitcast(F32)
    t1 = pool.tile([128, n], F32, tag=f"rs_t1{n}", name="t1")
    nc.vector.tensor_tensor(out=t1, in0=y0, in1=y0, op=OP.mult)      # y0^2
    t2 = pool.tile([128, n], F32, tag=f"rs_t2{n}", name="t2")
    nc.vector.scalar_tensor_tensor(
        out=t2, in0=ve, scalar=-0.5, in1=t1, op0=OP.mult, op1=OP.mult
    )                                                                # -v y0^2/2
    nc.vector.tensor_scalar(
        out=t2, in0=t2, scalar1=1.5, scalar2=None, op0=OP.add
    )                                                                # 1.5 - v y0^2/2
    nc.vector.tensor_tensor(out=out_f16, in0=y0, in1=t2, op=OP.mult)


def _emit(ctx: ExitStack, tc: tile.TileContext):
    nc = tc.nc
    xt_d = nc.dram_tensor("xt", [D, S], F16, kind="ExternalInput").ap()
    expa_d = nc.dram_tensor("expa", [KA * 128, QH], F16, kind="ExternalInput").ap()
    cmask_d = nc.dram_tensor("cmask", [KB * 128, QH], F16, kind="ExternalInput").ap()
    logm_d = nc.dram_tensor("logm", [KC * 128, QH], F16, kind="ExternalInput").ap()
    wq_d = nc.dram_tensor("wq", [D, D], F16, kind="ExternalInput").ap()
    wk_d = nc.dram_tensor("wk", [D, D], F16, kind="ExternalInput").ap()
    wv_d = nc.dram_tensor("wv", [D, D], F16, kind="ExternalInput").ap()
    wo_d = nc.dram_tensor("wo", [D, D], F16, kind="ExternalInput").ap()
    w1_d = nc.dram_tensor("w1", [D, 4 * D], F16, kind="ExternalInput").ap()
    w2_d = nc.dram_tensor("w2", [4 * D, D], F16, kind="ExternalInput").ap()
    # blob cols: 0 bias_q, 1 bias_k, 2 bias_o, 3 bias_2
    blob_d = nc.dram_tensor("blob", [D, 4], F32, kind="ExternalInput").ap()
    b1_d = nc.dram_tensor("b1", [4 * D, 1], F32, kind="ExternalInput").ap()
    # crow rows: 0 -cq, 1 -ck, 2 -cv, 3 bias_v
    crow_d = nc.dram_tensor("crow", [4, D], F16, kind="ExternalInput").ap()
    ones_d = nc.dram_tensor("onesrow", [1, S], F16, kind="ExternalInput").ap()
    out_d = nc.dram_tensor("out", [QH, D], F16, kind="ExternalOutput").ap()

    consts = ctx.enter_context(tc.tile_pool(name="consts", bufs=1))
    main = ctx.enter_context(tc.tile_pool(name="main", bufs=1))
    small = ctx.enter_context(tc.tile_pool(name="small", bufs=2))
    # PSUM: 3x 2-bank "big" + 2x 1-bank "sm" = 8 banks exactly
    ps_big = ctx.enter_context(tc.tile_pool(name="ps_big", bufs=3, space="PSUM"))
    ps_sm = ctx.enter_context(tc.tile_pool(name="ps_sm", bufs=2, space="PSUM"))

    xt_q = consts.tile([128, 2, QH], F16, tag="xtq")
    wq_sb = consts.tile([128, 2, D], F16, tag="wq")
    wk_sb = consts.tile([128, 2, D], F16, tag="wk")
    wv_sb = consts.tile([128, 2, D], F16, tag="wv")
    wo_sb = consts.tile([128, 2, D], F16, tag="wo")
    w1_sb = consts.tile([128, 2, 4 * D], F16, tag="w1")
    w2_sb = consts.tile([128, 8, D], F16, tag="w2")
    blob_sb = consts.tile([128, 2, 4], F32, tag="blob")
    b1_sb = consts.tile([128, 8, 1], F32, tag="b1")
    cq_sb = consts.tile([1, D], F16, tag="cq")
    ck_sb = consts.tile([1, D], F16, tag="ck")
    cv_sb = consts.tile([1, D], F16, tag="cv")
    bv_sb = consts.tile([1, D], F16, tag="bv")
    ones_row = consts.tile([1, S], F16, tag="onesr")
    expa_sb = consts.tile([128, KA, QH], F16, tag="expa")
    cmask_sb = consts.tile([128, KB, QH], F16, tag="cmask")
    logm_sb = consts.tile([128, KC, QH], F16, tag="logm")
    identh = consts.tile([128, 128], F16, tag="identh")
    ones_col = consts.tile([128, 1], F16, tag="onesc")
    magic_sb = consts.tile([128, DT], I32, tag="magic")
    shift_sb = consts.tile([128, 1], F32, tag="shift")

    qt8 = main.tile([128, 2, 2, QH], F8, tag="qt8")
    kt8 = main.tile([128, 2, 2, S], F8, tag="kt8")
    vaug = main.tile([128, DT, H, HD + 1], F16, tag="vaug")
    r_row = main.tile([1, S], F16, tag="rrow")
    u_row = main.tile([1, S], F16, tag="urow")
    ctx_sb = main.tile([128, 8, D], F16, tag="ctxr")
    ctxT = main.tile([128, 2, QH], F16, tag="ctxT")
    yT = main.tile([128, 2, QH], F16, tag="yT")
    y2 = main.tile([128, 2, QH], F16, tag="y2")
    outT = main.tile([128, 2, QH], F16, tag="outT")
    out_sb = main.tile([128, 8, D], F16, tag="outr")
    ro2bc = main.tile([128, QH], F16, tag="ro2bc")
    u2_row = main.tile([1, QH], F16, tag="u2row")

    # ---------------- prolog DMAs and consts ----------------

    nc.sync.dma_start(out=blob_sb, in_=blob_d.rearrange("(t p) i -> p t i", p=128))
    nc.sync.dma_start(out=cq_sb, in_=crow_d[0:1])
    nc.sync.dma_start(out=ck_sb, in_=crow_d[1:2])
    nc.sync.dma_start(out=cv_sb, in_=crow_d[2:3])
    nc.sync.dma_start(out=bv_sb, in_=crow_d[3:4])
    nc.sync.dma_start(out=ones_row, in_=ones_d)
    nc.sync.dma_start(out=wq_sb, in_=wq_d.rearrange("(t p) m -> p t m", p=128))
    nc.sync.dma_start(out=wk_sb, in_=wk_d.rearrange("(t p) m -> p t m", p=128))
    nc.sync.dma_start(out=wv_sb, in_=wv_d.rearrange("(t p) m -> p t m", p=128))
    # per-kt mask DMAs in consumption order so kt0's tile lands first
    expa_r = expa_d.rearrange("(k p) q -> p k q", p=128)
    cmask_r = cmask_d.rearrange("(k p) q -> p k q", p=128)
    logm_r = logm_d.rearrange("(k p) q -> p k q", p=128)
    for kt in range(DT):
        fl = KT_FLAVOR[kt]
        if fl == "b":
            i = B_IDX[kt]
            nc.sync.dma_start(out=cmask_sb[:, i, :], in_=cmask_r[:, i, :])
        elif fl == "c":
            i = C_IDX[kt]
            nc.sync.dma_start(out=logm_sb[:, i, :], in_=logm_r[:, i, :])
        else:
            i = A_IDX[kt]
            nc.sync.dma_start(out=expa_sb[:, i, :], in_=expa_r[:, i, :])
    nc.sync.dma_start(out=wo_sb, in_=wo_d.rearrange("(t p) m -> p t m", p=128))
    nc.sync.dma_start(out=w1_sb, in_=w1_d.rearrange("(t p) m -> p t m", p=128))
    nc.sync.dma_start(out=w2_sb, in_=w2_d.rearrange("(t p) m -> p t m", p=128))
    nc.sync.dma_start(out=b1_sb, in_=b1_d.rearrange("(t p) i -> p t i", p=128))
    nc.sync.dma_start(
        out=xt_q, in_=xt_d.rearrange("(t p) s -> p t s", p=128)[:, :, 0:QH]
    )

    make_identity(nc, identh[:])
    nc.vector.memset(ones_col, 1.0)
    nc.vector.memset(magic_sb, int(0x5F3759DF))
    nc.vector.memset(shift_sb, SHIFT)
    nc.gpsimd.memset(qt8[:, :, 1, :], 0.0)
    nc.gpsimd.memset(kt8[:, :, 1, :], 0.0)
    nc.gpsimd.memset(vaug[:, :, :, HD : HD + 1], 1.0)

    with tc.tile_pool(name="prolog", bufs=1) as prolog:
        xt_sb = prolog.tile([128, 2, S], F16, tag="xtf")
        xsq = prolog.tile([128, 2, S], F16, tag="xsq")
        xs_sb = prolog.tile([128, 2, S], F16, tag="xs")
        robc = prolog.tile([128, S], F16, tag="robc")
        with tc.high_priority():
            xtr = xt_d.rearrange("(t p) s -> p t s", p=128)
            nc.sync.dma_start(out=xt_sb[:, :, 0:QH], in_=xtr[:, :, 0:QH])
            nc.sync.dma_start(out=xt_sb[:, :, QH:S], in_=xtr[:, :, QH:S])

        # ---- LN1 stats in column space, pipelined per token-half ----
        st_col = ps_sm.tile([128, 2, DT], F32, tag="sm")
        ru = small.tile([128, 2, DT], F16, tag="ru")
        for th in range(2):
            HCH = DT // 2
            ch0 = th * HCH
            for dt in range(2):
                nc.vector.tensor_tensor(
                    out=xsq[:, dt, th * QH : th * QH + QH],
                    in0=xt_sb[:, dt, th * QH : th * QH + QH],
                    in1=xt_sb[:, dt, th * QH : th * QH + QH],
                    op=OP.mult,
                )
            for q, srct in ((0, xt_sb), (1, xsq)):
                for ch in range(ch0, ch0 + HCH):
                    for dt in range(2):
                        nc.tensor.matmul(
                            st_col[:, q, ch : ch + 1],
                            (srct[:, dt, ch * 128 : ch * 128 + 128]),
                            (ones_col[:]),
                            start=(th == 0 and q == 0 and ch == 0 and dt == 0),
                            stop=(th == 1 and q == 1 and ch == DT - 1 and dt == 1),
                        )
            mv = small.tile([128, 2, HCH], F32, tag="mv")
            nc.vector.tensor_scalar(
                out=mv, in0=st_col[:, :, ch0 : ch0 + HCH], scalar1=1.0 / D,
                scalar2=None, op0=OP.mult,
            )
            musq = small.tile([128, HCH], F32, tag="musq")
            nc.vector.tensor_tensor(out=musq, in0=mv[:, 0, :], in1=mv[:, 0, :], op=OP.mult)
            var = small.tile([128, HCH], F32, tag="var")
            nc.vector.tensor_tensor(out=var, in0=mv[:, 1, :], in1=musq, op=OP.subtract)
            _rsqrt_dve(nc, small, var, ru[:, 0, ch0 : ch0 + HCH], HCH, magic_sb)
            nc.vector.tensor_tensor(
                out=ru[:, 1, ch0 : ch0 + HCH], in0=mv[:, 0, :],
                in1=ru[:, 0, ch0 : ch0 + HCH], op=OP.mult,
            )
            # transpose rho / u columns back to rows for this half
            for q, dst in ((0, r_row), (1, u_row)):
                rT = ps_big.tile([1, QH], F16, tag="big", name=f"rT{q}_{th}")
                for ch in range(ch0, ch0 + HCH):
                    nc.tensor.transpose(
                        rT[0:1, (ch - ch0) * 128 : (ch - ch0) * 128 + 128],
                        ru[:, q, ch : ch + 1],
                        identh[:],
                    )
                nc.vector.tensor_copy(out=dst[0:1, th * QH : th * QH + QH], in_=rT[0:1, :])
            pb = ps_big.tile([128, QH], F32, tag="big")
            for j in range(2):
                nc.tensor.matmul(
                    pb[:, j * 512 : j * 512 + 512], (ones_row[0:1, 0:128]),
                    (r_row[0:1, th * QH + j * 512 : th * QH + j * 512 + 512]),
                    start=True, stop=True,
                )
            nc.scalar.activation(
                out=robc[:, th * QH : th * QH + QH], in_=pb[:], func=AF.Copy
            )
            for dt in range(2):
                nc.vector.tensor_tensor(
                    out=xs_sb[:, dt, th * QH : th * QH + QH],
                    in0=xt_sb[:, dt, th * QH : th * QH + QH],
                    in1=robc[:, th * QH : th * QH + QH], op=OP.mult,
                )
        # ---------------- projections ----------------
        def qk_proj(dst, w_sb, bias_i, c_sb, g, ch, use_act):
            ps = ps_big.tile([128, QH], F32, tag="big")
            for j in range(2):
                t0, t1 = ch * QH + j * 512, ch * QH + j * 512 + 512
                for dt in range(2):
                    nc.tensor.matmul(
                        ps[:, j * 512 : j * 512 + 512],
                        (w_sb[:, dt, g * 128 : g * 128 + 128]),
                        (xs_sb[:, dt, t0:t1]),
                        start=(dt == 0), stop=False,
                    )
                nc.tensor.matmul(
                    ps[:, j * 512 : j * 512 + 512],
                    (c_sb[0:1, g * 128 : g * 128 + 128]),
                    (u_row[0:1, t0:t1]),
                    start=False, stop=True,
                )
            dstap = dst[:, g, 0, ch * QH : ch * QH + QH]
            if use_act:
                nc.scalar.activation(
                    out=dstap, in_=ps[:], func=AF.Identity,
                    bias=blob_sb[:, g, bias_i : bias_i + 1],
                )
            else:
                nc.vector.tensor_scalar(
                    out=dstap, in0=ps[:],
                    scalar1=blob_sb[:, g, bias_i : bias_i + 1],
                    scalar2=None, op0=OP.add,
                )

        for g in range(2):
            qk_proj(qt8, wq_sb, 0, cq_sb, g, 0, use_act=(g == 0))
        for g in range(2):
            for ch in range(2):
                qk_proj(kt8, wk_sb, 1, ck_sb, g, ch, use_act=((g + ch) % 2 == 0))

        def v_proj(st, use_act):
            ps = ps_sm.tile([128, D], F32, tag="sm")
            for dt in range(2):
                nc.tensor.matmul(
                    ps[:],
                    (xs_sb[:, dt, st * 128 : st * 128 + 128]),
                    (wv_sb[:, dt, :]),
                    start=(dt == 0), stop=False,
                )
            nc.tensor.matmul(
                ps[:], (u_row[0:1, st * 128 : st * 128 + 128]), (cv_sb[0:1, :]),
                start=False, stop=False,
            )
            nc.tensor.matmul(
                ps[:], (ones_row[0:1, st * 128 : st * 128 + 128]), (bv_sb[0:1, :]),
                start=False, stop=True,
            )
            dst = vaug[:, st, :, 0:HD]
            src = ps.rearrange("p (h f) -> p h f", h=H)
            if use_act:
                nc.scalar.activation(out=dst, in_=src, func=AF.Copy)
            else:
                nc.vector.tensor_copy(out=dst, in_=src)

        for st in range(DT):
            v_proj(st, use_act=(st % 2 == 0))

    # ---------------- attention ----------------
    def flavor(kt):
        return KT_FLAVOR[kt]

    with tc.tile_pool(name="expt", bufs=1) as expt_pool:
        ets = {}    # (h, kt) -> f16 AP [128, QH]

        def qk_step(h, kt):
            g, base = h // 4, 32 * (h % 4)
            fl = flavor(kt)
            ps = ps_big.tile([128, QH], F32, tag="big")
            for qb in range(2):
                nc.tensor.matmul(
                    ps[:, qb * 512 : qb * 512 + 512],
                    (kt8[base : base + 32, g, :, kt * 128 : kt * 128 + 128]),
                    (qt8[base : base + 32, g, :, qb * 512 : qb * 512 + 512]),
                    start=True, stop=(fl != "c"),
                    perf_mode=PM.DoubleRow,
                    tile_position=(base, 0),
                )
            if fl == "c":
                for qb in range(2):
                    nc.tensor.matmul(
                        ps[:, qb * 512 : qb * 512 + 512],
                        (identh[:]),
                        (logm_sb[:, C_IDX[kt], qb * 512 : qb * 512 + 512]),
                        start=False, stop=True,
                    )
            if fl == "b":
                exi = expt_pool.tile(
                    [128, QH], I16, tag="eb", bufs=2 * NB + 2, name=f"eb{h}_{kt}"
                )
                nc.vector.scalar_tensor_tensor(
                    out=exi, in0=ps[:], scalar=ASCH,
                    in1=cmask_sb[:, B_IDX[kt], :], op0=OP.mult, op1=OP.add,
                )
                ets[(h, kt)] = exi.bitcast(F16)
            elif fl == "c":
                ea = expt_pool.tile(
                    [128, QH], F16, tag="ea2", bufs=2 * (NA2 + NA + NC_F) + 2,
                    name=f"ec{h}_{kt}"
                )
                nc.scalar.activation(out=ea, in_=ps[:], func=AF.Exp, bias=shift_sb)
                ets[(h, kt)] = ea[:]
            elif fl == "a2":
                ea = expt_pool.tile(
                    [128, QH], F16, tag="ea2", bufs=2 * (NA2 + NA + NC_F) + 2, name=f"ea2_{h}_{kt}"
                )
                nc.scalar.activation(out=ea, in_=ps[:], func=AF.Exp, bias=shift_sb)
                nc.gpsimd.tensor_tensor(
                    out=ea, in0=ea, in1=expa_sb[:, A_IDX[kt], :], op=OP.mult
                )
                ets[(h, kt)] = ea[:]
            else:
                ea = expt_pool.tile(
                    [128, QH], F16, tag="ea2", bufs=2 * (NA2 + NA + NC_F) + 2,
                    name=f"ea{h}_{kt}"
                )
                nc.scalar.activation(out=ea, in_=ps[:], func=AF.Exp, bias=shift_sb)
                nc.vector.tensor_tensor(
                    out=ea, in0=ea, in1=expa_sb[:, A_IDX[kt], :], op=OP.mult
                )
                ets[(h, kt)] = ea[:]

        pv_tiles = {}

        def pv_part(h, qtg, qi):
            # one query-tile's 16-kt accumulation chain (spread across steps)
            if (h, qtg) not in pv_tiles:
                pv_tiles[(h, qtg)] = ps_sm.tile(
                    [128, 4, HD + 1], F32, tag="sm", name=f"pv{h}_{qtg}"
                )
            pv = pv_tiles[(h, qtg)]
            qt = qtg * 4 + qi
            for kt in range(DT):
                nc.tensor.matmul(
                    pv[:, qi, :],
                    ets[(h, kt)][:, qt * 128 : qt * 128 + 128],
                    vaug[:, kt, h, :],
                    start=(qi == 0 and kt == 0),
                    stop=(qi == 3 and kt == DT - 1),
                )

        def pv_drain(h, qtg):
            pv = pv_tiles.pop((h, qtg))
            dn = small.tile([128, 4], F32, tag="dn")
            nc.vector.reciprocal(out=dn, in_=pv[:, :, HD])
            nc.vector.scalar_tensor_tensor(
                out=ctx_sb[:, qtg * 4 : qtg * 4 + 4, h * HD : h * HD + HD],
                in0=pv[:, :, 0:HD], scalar=1.0,
                in1=dn.unsqueeze(2).broadcast_to([128, 4, HD]),
                op0=OP.mult, op1=OP.mult,
            )
            if qtg == 1:
                for kt in range(DT):
                    del ets[(h, kt)]

        def pv_group(h, qtg):
            for qi in range(4):
                pv_part(h, qtg, qi)
            pv_drain(h, qtg)

        pv7 = {}
        for h in range(H):
            for kt in range(DT):
                qk_step(h, kt)
                if h > 0 and 2 <= kt <= 5:
                    pv_part(h - 1, 0, kt - 2)
                if h > 0 and kt == 6:
                    pv_drain(h - 1, 0)
                if h > 0 and 8 <= kt <= 11:
                    pv_part(h - 1, 1, kt - 8)
                if h > 0 and kt == 12:
                    pv_drain(h - 1, 1)
                if h == H - 1:
                    # kt-major PV for the last head: accumulate as ets arrive
                    if "t" not in pv7:
                        pv7["t"] = ps_sm.tile(
                            [128, 8, HD + 1], F32, tag="sm", name="pv7"
                        )
                    for qt in range(8):
                        nc.tensor.matmul(
                            pv7["t"][:, qt, :],
                            ets[(h, kt)][:, qt * 128 : qt * 128 + 128],
                            vaug[:, kt, h, :],
                            start=(qt == 0 and kt == 0),
                            stop=(qt == 7 and kt == DT - 1),
                        )

        def pv7_drain(qtg):
            pv = pv7["t"]
            dn = small.tile([128, 4], F32, tag="dn")
            nc.vector.reciprocal(out=dn, in_=pv[:, qtg * 4 : qtg * 4 + 4, HD])
            nc.vector.scalar_tensor_tensor(
                out=ctx_sb[:, qtg * 4 : qtg * 4 + 4, (H - 1) * HD : H * HD],
                in0=pv[:, qtg * 4 : qtg * 4 + 4, 0:HD], scalar=1.0,
                in1=dn.unsqueeze(2).broadcast_to([128, 4, HD]),
                op0=OP.mult, op1=OP.mult,
            )
            if qtg == 1:
                for kt in range(DT):
                    if (H - 1, kt) in ets:
                        del ets[(H - 1, kt)]
        pv7_drain(0)

        # -------- post-attention per 512-token chunk --------
        def post_a(qb):
            t0 = qb * 512
            for dt in range(2):
                pt = ps_sm.tile([128, 512], F16, tag="sm")
                for qq in range(4):
                    qt = qb * 4 + qq
                    nc.tensor.transpose(
                        pt[:, qq * 128 : qq * 128 + 128],
                        ctx_sb[:, qt, dt * 128 : dt * 128 + 128],
                        identh[:],
                    )
                if dt == 0:
                    nc.scalar.activation(
                        out=ctxT[:, dt, t0 : t0 + 512], in_=pt[:], func=AF.Copy
                    )
                else:
                    nc.vector.tensor_copy(out=ctxT[:, dt, t0 : t0 + 512], in_=pt[:])
            for mt in range(2):
                ps = ps_big.tile([128, QH], F32, tag="big")
                for dt in range(2):
                    nc.tensor.matmul(
                        ps[:, 0:512],
                        (wo_sb[:, dt, mt * 128 : mt * 128 + 128]),
                        (ctxT[:, dt, t0 : t0 + 512]),
                        start=(dt == 0), stop=(dt == 1),
                    )
                nc.vector.scalar_tensor_tensor(
                    out=yT[:, mt, t0 : t0 + 512], in0=ps[:, 0:512],
                    scalar=blob_sb[:, mt, 2:3], in1=xt_q[:, mt, t0 : t0 + 512],
                    op0=OP.add, op1=OP.add,
                )
            ysq = small.tile([128, 2, 512], F16, tag="ysq")
            for dt in range(2):
                nc.vector.tensor_tensor(
                    out=ysq[:, dt, :], in0=yT[:, dt, t0 : t0 + 512],
                    in1=yT[:, dt, t0 : t0 + 512], op=OP.mult,
                )
            st2 = ps_sm.tile([128, 2, 4], F32, tag="sm")
            for q in range(2):
                for ch in range(4):
                    for dt in range(2):
                        src = (
                            yT[:, dt, t0 + ch * 128 : t0 + ch * 128 + 128]
                            if q == 0
                            else ysq[:, dt, ch * 128 : ch * 128 + 128]
                        )
                        nc.tensor.matmul(
                            st2[:, q, ch : ch + 1], (src), (ones_col[:]),
                            start=(q == 0 and ch == 0 and dt == 0),
                            stop=(q == 1 and ch == 3 and dt == 1),
                        )
            mv2 = small.tile([128, 2, 4], F32, tag="mv2")
            nc.vector.tensor_scalar(
                out=mv2, in0=st2, scalar1=1.0 / D, scalar2=None, op0=OP.mult
            )
            ms2 = small.tile([128, 4], F32, tag="ms2")
            nc.vector.tensor_tensor(out=ms2, in0=mv2[:, 0, :], in1=mv2[:, 0, :], op=OP.mult)
            var2 = small.tile([128, 4], F32, tag="var2")
            nc.vector.tensor_tensor(out=var2, in0=mv2[:, 1, :], in1=ms2, op=OP.subtract)
            ru2 = small.tile([128, 2, 4], F16, tag="ru2")
            _rsqrt_dve(nc, small, var2, ru2[:, 0, :], 4, magic_sb)
            nc.vector.tensor_tensor(
                out=ru2[:, 1, :], in0=mv2[:, 0, :], in1=ru2[:, 0, :], op=OP.mult
            )
            rows2 = ps_sm.tile([1, 2, 512], F16, tag="sm")
            for q in range(2):
                for ch in range(4):
                    nc.tensor.transpose(
                        rows2[0:1, q, ch * 128 : ch * 128 + 128],
                        ru2[:, q, ch : ch + 1],
                        identh[:],
                    )
            r2r = small.tile([1, 512], F16, tag="r2r")
            nc.vector.tensor_copy(out=r2r, in_=rows2[0:1, 0, :])
            nc.vector.tensor_copy(out=u2_row[0:1, t0 : t0 + 512], in_=rows2[0:1, 1, :])
            pb = ps_big.tile([128, QH], F32, tag="big")
            nc.tensor.matmul(
                pb[:, 0:512], (ones_row[0:1, 0:128]), (r2r[:]), start=True, stop=True
            )
            nc.tensor.matmul(
                pb[:, 512:1024], (ones_row[0:1, 0:128]),
                (u2_row[0:1, t0 : t0 + 512]), start=True, stop=True,
            )
            nc.scalar.activation(
                out=ro2bc[:, t0 : t0 + 512], in_=pb[:, 0:512], func=AF.Copy
            )
            u2bc = small.tile([128, 512], F16, tag="u2bc")
            nc.vector.tensor_copy(out=u2bc, in_=pb[:, 512:1024])
            for dt in range(2):
                nc.vector.tensor_tensor(
                    out=y2[:, dt, t0 : t0 + 512], in0=yT[:, dt, t0 : t0 + 512],
                    in1=ro2bc[:, t0 : t0 + 512], op=OP.mult,
                )
                nc.vector.tensor_tensor(
                    out=y2[:, dt, t0 : t0 + 512], in0=y2[:, dt, t0 : t0 + 512],
                    in1=u2bc, op=OP.subtract,
                )
        def post_b(qb):
            t0 = qb * 512
            with tc.tile_pool(name=f"mlp{qb}", bufs=1) as mlp_pool:
                hT = mlp_pool.tile([128, 8, 512], F16, tag="hT")
                for mt in range(8):
                    ps = ps_big.tile([128, QH], F32, tag="big")
                    for dt in range(2):
                        nc.tensor.matmul(
                            ps[:, 0:512],
                            (w1_sb[:, dt, mt * 128 : mt * 128 + 128]),
                            (y2[:, dt, t0 : t0 + 512]),
                            start=(dt == 0), stop=(dt == 1),
                        )
                    nc.scalar.activation(
                        out=hT[:, mt, :], in_=ps[:, 0:512],
                        func=AF.Gelu, bias=b1_sb[:, mt, :],
                    )
                ps2 = [ps_big.tile([128, QH], F32, tag="big", name=f"m2_{qb}_{m}")
                       for m in range(2)]
                for j in range(8):
                    for m in range(2):
                        nc.tensor.matmul(
                            ps2[m][:, 0:512],
                            (w2_sb[:, j, m * 128 : m * 128 + 128]),
                            (hT[:, j, :]),
                            start=(j == 0), stop=(j == 7),
                        )
                for m in range(2):
                    nc.vector.scalar_tensor_tensor(
                        out=outT[:, m, t0 : t0 + 512], in0=ps2[m][:, 0:512],
                        scalar=blob_sb[:, m, 3:4], in1=yT[:, m, t0 : t0 + 512],
                        op0=OP.add, op1=OP.add,
                    )
            for ch in range(4):
                po = ps_sm.tile([128, D], F16, tag="sm")
                for mt in range(2):
                    nc.tensor.transpose(
                        po[:, mt * 128 : mt * 128 + 128],
                        outT[:, mt, t0 + ch * 128 : t0 + ch * 128 + 128],
                        identh[:],
                    )
                gi = qb * 4 + ch
                if ch % 2 == 0:
                    nc.scalar.activation(out=out_sb[:, gi, :], in_=po[:], func=AF.Copy)
                else:
                    nc.vector.tensor_copy(out=out_sb[:, gi, :], in_=po[:])
                nc.sync.dma_start(
                    out=out_d.rearrange("(g p) d -> p g d", p=128)[:, gi, :],
                    in_=out_sb[:, gi, :],
                )

        post_a(0)
        pv7_drain(1)
        post_a(1)
        post_b(0)
        post_b(1)


_NC_CACHE = {}


def _get_nc():
    if "nc" not in _NC_CACHE:
        nc = bacc.Bacc("TRN2", target_bir_lowering=False, debug=False)
        with tile.TileContext(nc) as tc:
            with ExitStack() as ctx:
                _emit(ctx, tc)
        nc.compile()
        _NC_CACHE["nc"] = nc
    return _NC_CACHE["nc"]


def _prep_common(inputs):
    f = lambda k: np.asarray(inputs[k], np.float32)
    g1, b1n = f("ln1_g"), f("ln1_b")
    g2, b2n = f("ln2_g"), f("ln2_b")
    wq, wk, wv, wo = f("wq"), f("wk"), f("wv"), f("wo")
    w1, w2 = f("w1"), f("w2")

    wq_eff = (wq * g1[None, :]).T * SCALE
    wk_eff = (wk * g1[None, :]).T
    wv_eff = (wv * g1[None, :]).T
    w1_eff = (w1 * g2[None, :]).T
    bias_q = (f("bq") + wq @ b1n) * SCALE
    bias_k = f("bk") + wk @ b1n
    bias_v = f("bv") + wv @ b1n
    bias_1 = f("b1") + w1 @ b2n
    blob = np.stack([bias_q, bias_k, f("bo"), f("b2")], axis=1)
    crow = np.stack(
        [-wq_eff.sum(0), -wk_eff.sum(0), -wv_eff.sum(0), bias_v], axis=0
    )
    return {
        "wq": np.ascontiguousarray(wq_eff).astype(np.float16),
        "wk": np.ascontiguousarray(wk_eff).astype(np.float16),
        "wv": np.ascontiguousarray(wv_eff).astype(np.float16),
        "wo": np.ascontiguousarray(wo.T).astype(np.float16),
        "w1": np.ascontiguousarray(w1_eff).astype(np.float16),
        "w2": np.ascontiguousarray(w2.T).astype(np.float16),
        "blob": np.ascontiguousarray(blob).astype(np.float32),
        "b1": bias_1.reshape(4 * D, 1).astype(np.float32),
        "crow": np.ascontiguousarray(crow).astype(np.float16),
        "onesrow": np.ones((1, S), np.float16),
    }


def _run(inputs, trace=False):
    x = np.asarray(inputs["x"], np.float32)
    adj = np.asarray(inputs["adj_mask"]).astype(bool)
    ea = np.asarray(inputs["edge_attr"], np.float32).reshape(-1)
    ei = np.asarray(inputs["edge_index"]).astype(np.int64)

    bias2d = np.zeros((S, S), np.float32)
    bias2d[ei[0], ei[1]] = np.clip(ea, -5.0, 5.0)

    common = _prep_common(inputs)
    in_maps = []
    for c in range(NCORES):
        b, qh = c // 2, c % 2
        ordr = np.arange(S)
        if qh == 1:
            ordr = np.concatenate([ordr[QH:], ordr[:QH]])
        xc = x[b][ordr]
        adj_kq = adj[b][ordr[:QH]][:, ordr].T      # [k rotated, q(1024)]
        bias_kq = bias2d[ordr[:QH]][:, ordr].T
        expa = np.where(adj_kq, np.exp(bias_kq), 0.0).astype(np.float16)
        cm = np.where(
            adj_kq, ASCH * (bias_kq + SHIFT) + BSCH, -60000.0
        ).astype(np.float16)
        lg = np.where(adj_kq, bias_kq, -30.0).astype(np.float16)
        er = expa.reshape(DT, 128, QH)
        cr = cm.reshape(DT, 128, QH)
        lr = lg.reshape(DT, 128, QH)
        in_maps.append(
            {
                "xt": np.ascontiguousarray(xc.T).astype(np.float16),
                "expa": np.ascontiguousarray(
                    er[AKTS].reshape(KA * 128, QH)
                ),
                "cmask": np.ascontiguousarray(
                    cr[BKTS].reshape(KB * 128, QH)
                ),
                "logm": np.ascontiguousarray(
                    lr[CKTS].reshape(KC * 128, QH)
                ),
                **common,
            }
        )

    nc = _get_nc()
    res = run_bass_kernel_spmd(nc, in_maps, core_ids=list(range(NCORES)), trace=trace)
    outs = [res.results[c]["out"].astype(np.float32) for c in range(NCORES)]
    y = np.stack(
        [np.concatenate([outs[2 * b], outs[2 * b + 1]], axis=0) for b in range(B)],
        axis=0,
    )
    return y, res


def kernel(**inputs) -> np.ndarray:
    y, _ = _run(inputs, trace=False)
    return y


# revision 5
# speedup vs baseline: 1.0102x; 1.0102x over previous
"""NodeSetAttention TRN2 Bass kernel (fully-transposed dataflow).

Shapes: B=4, S=2048, D=256, H=8, HD=32, E=65536. 8 cores: core c ->
(batch b=c//2, query half qh=c%2); tokens are rotated so the core's 1024
queries are columns 0..1023 and the keys are all 2048 columns.

Design highlights (timing target = instruction_cost_v2 / TimelineSim):
- Everything stays transposed [feature, token]. LN stats come from PE
  column-sum matmuls (free size 1) finished in [128, DT] column space on
  DVE (incl. a Quake-style rsqrt bit trick + one Newton step), then
  transposed back to rho/u rows. LN affines fold into the weights and
  biases host-side; the -mu*rho correction rides each projection as a
  K=1 augmentation pass against the u row; rho itself scales the tokens
  once (xs = xT * rho_bcast).
- QK runs fp8e4m3 DoubleRow (0.5 cycles/row): q/k are [128, grp, slot,
  token] with slot1 zeroed (the zero slot makes tight head packing legal
  at tile_position bases 0/32/64/96). Head h uses partitions
  32*(h%4)..+32 of group h//4.
- The masked softmax runs as exp(scores + bias + SHIFT) * mask with four
  per-kt flavors, interleaved so all four engines stream concurrently:
    a2: ACT exp, GPSIMD tensor_tensor mask-mul
    a:  ACT exp, DVE tensor_tensor mask-mul (2x f16 mode)
    b:  DVE fused Schraudolph bit-trick exp: i16 = psum*ASCH + C,
        bitcast to f16. C folds mask, bias, shift and BSCH; masked-out
        lanes saturate to -0.0. Verified in (-10.35, 11.83) safe window.
    c:  PE identity-matmul accumulates log-mask into the QK psum, then a
        plain ACT exp (no elementwise mask op at all).
- PV in fp16 with a ones column for the softmax denominator; per-4-qt
  psum groups, drained with reciprocal + a stride-0-broadcast stt.
- Post-attention (O + LN2 + MLP) per 512-token chunk, the first chunk
  overlapping the last head's PV; MLP1 pre-subtracts u2 via a broadcast,
  gelu+biases fused into the ACT drains; outputs transposed back via PE
  and DMA'd per 128-row group.
"""

import math
from contextlib import ExitStack

import numpy as np

import concourse.bass as bass
import concourse.bacc as bacc
import concourse.mybir as mybir
import concourse.tile as tile
from concourse.bass_utils import run_bass_kernel_spmd
from concourse.masks import make_identity

B, S, D, H, E = 4, 2048, 256, 8, 65536
HD = D // H
QH = S // 2
NCORES = 8
DT = S // 128
F32 = mybir.dt.float32
F16 = mybir.dt.float16
F8 = mybir.dt.float8e4
I16 = mybir.dt.int16
AF = mybir.ActivationFunctionType
OP = mybir.AluOpType
PM = mybir.MatmulPerfMode

SHIFT = 1.0
ASCH = float(2**10 / math.log(2.0))
BSCH = 15301.0
SCALE = 1.0 / math.sqrt(HD)

# per-kt exp flavor: a2 = ACT exp + GPSIMD mul, a = ACT exp + DVE mul,
# b = DVE fused Schraudolph, c = PE identity-add of log-mask + ACT exp.
# Interleaved so all four engines run concurrently.
KT_FLAVOR = ["b", "a2", "c", "b", "a2", "a", "b", "a2", "c", "a", "b",
             "a2", "a", "b", "a2", "b"]
CKTS = [kt for kt, f in enumerate(KT_FLAVOR) if f == "c"]
C_IDX = {kt: i for i, kt in enumerate(CKTS)}
KC = len(CKTS)
NC_F = KC
AKTS = [kt for kt, f in enumerate(KT_FLAVOR) if f in ("a", "a2")]

BKTS = [kt for kt, f in enumerate(KT_FLAVOR) if f == "b"]
A_IDX = {kt: i for i, kt in enumerate(AKTS)}
B_IDX = {kt: i for i, kt in enumerate(BKTS)}
KA = len(AKTS)
KB = len(BKTS)
NA2 = sum(1 for f in KT_FLAVOR if f == "a2")
NA = sum(1 for f in KT_FLAVOR if f == "a")
NB = KB


I32 = mybir.dt.int32


def _rsqrt_dve(nc, pool, var, out_f16, n, magic_sb):
    """out = 1/sqrt(var + eps) entirely on DVE: Quake bit trick + 1 Newton
    step (rel err ~2e-3, fine vs the 2e-2 gate). All ops tiny [128, n]."""
    ve = pool.tile([128, n], F32, tag=f"rs_ve{n}", name="ve")
    nc.vector.tensor_scalar(
        out=ve, in0=var, scalar1=1e-5, scalar2=None, op0=OP.add
    )
    sh = pool.tile([128, n], I32, tag=f"rs_sh{n}", name="sh")
    nc.vector.tensor_scalar(
        out=sh, in0=ve.bitcast(I32), scalar1=1, scalar2=None,
        op0=OP.logical_shift_right,
    )
    yi = pool.tile([128, n], I32, tag=f"rs_yi{n}", name="yi")
    nc.vector.tensor_tensor(
        out=yi, in0=magic_sb[:, 0:n].bitcast(I32), in1=sh, op=OP.subtract
    )
    y0 = yi.bitcast(F32)
    t1 = pool.tile([128, n], F32, tag=f"rs_t1{n}", name="t1")
    nc.vector.tensor_tensor(out=t1, in0=y0, in1=y0, op=OP.mult)      # y0^2
    t2 = pool.tile([128, n], F32, tag=f"rs_t2{n}", name="t2")
    nc.vector.scalar_tensor_tensor(
        out=t2, in0=ve, scalar=-0.5, in1=t1, op0=OP.mult, op1=OP.mult
    )                                                                # -v y0^2/2
    nc.vector.tensor_scalar(
        out=t2, in0=t2, scalar1=1.5, scalar2=None, op0=OP.add
    )                                                                # 1.5 - v y0^2/2
    nc.vector.tensor_tensor(out=out_f16, in0=y0, in1=t2, op=OP.mult)


def _emit(ctx: ExitStack, tc: tile.TileContext):
    nc = tc.nc
    xt_d = nc.dram_tensor("xt", [D, S], F16, kind="ExternalInput").ap()
    expa_d = nc.dram_tensor("expa", [KA * 128, QH], F16, kind="ExternalInput").ap()
    cmask_d = nc.dram_tensor("cmask", [KB * 128, QH], F16, kind="ExternalInput").ap()
    logm_d = nc.dram_tensor("logm", [KC * 128, QH], F16, kind="ExternalInput").ap()
    wq_d = nc.dram_tensor("wq", [D, D], F16, kind="ExternalInput").ap()
    wk_d = nc.dram_tensor("wk", [D, D], F16, kind="ExternalInput").ap()
    wv_d = nc.dram_tensor("wv", [D, D], F16, kind="ExternalInput").ap()
    wo_d = nc.dram_tensor("wo", [D, D], F16, kind="ExternalInput").ap()
    w1_d = nc.dram_tensor("w1", [D, 4 * D], F16, kind="ExternalInput").ap()
    w2_d = nc.dram_tensor("w2", [4 * D, D], F16, kind="ExternalInput").ap()
    # blob cols: 0 bias_q, 1 bias_k, 2 bias_o, 3 bias_2
    blob_d = nc.dram_tensor("blob", [D, 4], F32, kind="ExternalInput").ap()
    b1_d = nc.dram_tensor("b1", [4 * D, 1], F32, kind="ExternalInput").ap()
    # crow rows: 0 -cq, 1 -ck, 2 -cv, 3 bias_v
    crow_d = nc.dram_tensor("crow", [4, D], F16, kind="ExternalInput").ap()
    ones_d = nc.dram_tensor("onesrow", [1, S], F16, kind="ExternalInput").ap()
    out_d = nc.dram_tensor("out", [QH, D], F16, kind="ExternalOutput").ap()

    consts = ctx.enter_context(tc.tile_pool(name="consts", bufs=1))
    main = ctx.enter_context(tc.tile_pool(name="main", bufs=1))
    small = ctx.enter_context(tc.tile_pool(name="small", bufs=2))
    # PSUM: 3x 2-bank "big" + 2x 1-bank "sm" = 8 banks exactly
    ps_big = ctx.enter_context(tc.tile_pool(name="ps_big", bufs=3, space="PSUM"))
    ps_sm = ctx.enter_context(tc.tile_pool(name="ps_sm", bufs=2, space="PSUM"))

    xt_q = consts.tile([128, 2, QH], F16, tag="xtq")
    wq_sb = consts.tile([128, 2, D], F16, tag="wq")
    wk_sb = consts.tile([128, 2, D], F16, tag="wk")
    wv_sb = consts.tile([128, 2, D], F16, tag="wv")
    wo_sb = consts.tile([128, 2, D], F16, tag="wo")
    w1_sb = consts.tile([128, 2, 4 * D], F16, tag="w1")
    w2_sb = consts.tile([128, 8, D], F16, tag="w2")
    blob_sb = consts.tile([128, 2, 4], F32, tag="blob")
    b1_sb = consts.tile([128, 8, 1], F32, tag="b1")
    cq_sb = consts.tile([1, D], F16, tag="cq")
    ck_sb = consts.tile([1, D], F16, tag="ck")
    cv_sb = consts.tile([1, D], F16, tag="cv")
    bv_sb = consts.tile([1, D], F16, tag="bv")
    ones_row = consts.tile([1, S], F16, tag="onesr")
    expa_sb = consts.tile([128, KA, QH], F16, tag="expa")
    cmask_sb = consts.tile([128, KB, QH], F16, tag="cmask")
    logm_sb = consts.tile([128, KC, QH], F16, tag="logm")
    identh = consts.tile([128, 128], F16, tag="identh")
    ones_col = consts.tile([128, 1], F16, tag="onesc")
    magic_sb = consts.tile([128, DT], I32, tag="magic")
    shift_sb = consts.tile([128, 1], F32, tag="shift")

    qt8 = main.tile([128, 2, 2, QH], F8, tag="qt8")
    kt8 = main.tile([128, 2, 2, S], F8, tag="kt8")
    vaug = main.tile([128, DT, H, HD + 1], F16, tag="vaug")
    r_row = main.tile([1, S], F16, tag="rrow")
    u_row = main.tile([1, S], F16, tag="urow")
    ctx_sb = main.tile([128, 8, D], F16, tag="ctxr")
    ctxT = main.tile([128, 2, QH], F16, tag="ctxT")
    yT = main.tile([128, 2, QH], F16, tag="yT")
    y2 = main.tile([128, 2, QH], F16, tag="y2")
    outT = main.tile([128, 2, QH], F16, tag="outT")
    out_sb = main.tile([128, 8, D], F16, tag="outr")
    ro2bc = main.tile([128, QH], F16, tag="ro2bc")
    u2_row = main.tile([1, QH], F16, tag="u2row")

    # ---------------- prolog DMAs and consts ----------------

    nc.sync.dma_start(out=blob_sb, in_=blob_d.rearrange("(t p) i -> p t i", p=128))
    nc.sync.dma_start(out=cq_sb, in_=crow_d[0:1])
    nc.sync.dma_start(out=ck_sb, in_=crow_d[1:2])
    nc.sync.dma_start(out=cv_sb, in_=crow_d[2:3])
    nc.sync.dma_start(out=bv_sb, in_=crow_d[3:4])
    nc.sync.dma_start(out=ones_row, in_=ones_d)
    nc.sync.dma_start(out=wq_sb, in_=wq_d.rearrange("(t p) m -> p t m", p=128))
    nc.sync.dma_start(out=wk_sb, in_=wk_d.rearrange("(t p) m -> p t m", p=128))
    nc.sync.dma_start(out=wv_sb, in_=wv_d.rearrange("(t p) m -> p t m", p=128))
    # per-kt mask DMAs in consumption order so kt0's tile lands first
    expa_r = expa_d.rearrange("(k p) q -> p k q", p=128)
    cmask_r = cmask_d.rearrange("(k p) q -> p k q", p=128)
    logm_r = logm_d.rearrange("(k p) q -> p k q", p=128)
    for kt in range(DT):
        fl = KT_FLAVOR[kt]
        if fl == "b":
            i = B_IDX[kt]
            nc.sync.dma_start(out=cmask_sb[:, i, :], in_=cmask_r[:, i, :])
        elif fl == "c":
            i = C_IDX[kt]
            nc.sync.dma_start(out=logm_sb[:, i, :], in_=logm_r[:, i, :])
        else:
            i = A_IDX[kt]
            nc.sync.dma_start(out=expa_sb[:, i, :], in_=expa_r[:, i, :])
    nc.sync.dma_start(out=wo_sb, in_=wo_d.rearrange("(t p) m -> p t m", p=128))
    nc.sync.dma_start(out=w1_sb, in_=w1_d.rearrange("(t p) m -> p t m", p=128))
    nc.sync.dma_start(out=w2_sb, in_=w2_d.rearrange("(t p) m -> p t m", p=128))
    nc.sync.dma_start(out=b1_sb, in_=b1_d.rearrange("(t p) i -> p t i", p=128))
    nc.sync.dma_start(
        out=xt_q, in_=xt_d.rearrange("(t p) s -> p t s", p=128)[:, :, 0:QH]
    )

    make_identity(nc, identh[:])
    nc.vector.memset(ones_col, 1.0)
    nc.vector.memset(magic_sb, int(0x5F3759DF))
    nc.vector.memset(shift_sb, SHIFT)
    nc.gpsimd.memset(qt8[:, :, 1, :], 0.0)
    nc.gpsimd.memset(kt8[:, :, 1, :], 0.0)
    nc.gpsimd.memset(vaug[:, :, :, HD : HD + 1], 1.0)

    with tc.tile_pool(name="prolog", bufs=1) as prolog:
        xt_sb = prolog.tile([128, 2, S], F16, tag="xtf")
        xsq = prolog.tile([128, 2, S], F16, tag="xsq")
        xs_sb = prolog.tile([128, 2, S], F16, tag="xs")
        robc = prolog.tile([128, S], F16, tag="robc")
        with tc.high_priority():
            xtr = xt_d.rearrange("(t p) s -> p t s", p=128)
            nc.sync.dma_start(out=xt_sb[:, :, 0:QH], in_=xtr[:, :, 0:QH])
            nc.sync.dma_start(out=xt_sb[:, :, QH:S], in_=xtr[:, :, QH:S])

        # ---- LN1 stats in column space, pipelined per token-half ----
        st_col = ps_sm.tile([128, 2, DT], F32, tag="sm")
        ru = small.tile([128, 2, DT], F16, tag="ru")
        for th in range(2):
            HCH = DT // 2
            ch0 = th * HCH
            for dt in range(2):
                nc.vector.tensor_tensor(
                    out=xsq[:, dt, th * QH : th * QH + QH],
                    in0=xt_sb[:, dt, th * QH : th * QH + QH],
                    in1=xt_sb[:, dt, th * QH : th * QH + QH],
                    op=OP.mult,
                )
            for q, srct in ((0, xt_sb), (1, xsq)):
                for ch in range(ch0, ch0 + HCH):
                    for dt in range(2):
                        nc.tensor.matmul(
                            st_col[:, q, ch : ch + 1],
                            (srct[:, dt, ch * 128 : ch * 128 + 128]),
                            (ones_col[:]),
                            start=(th == 0 and q == 0 and ch == 0 and dt == 0),
                            stop=(th == 1 and q == 1 and ch == DT - 1 and dt == 1),
                        )
            mv = small.tile([128, 2, HCH], F32, tag="mv")
            nc.vector.tensor_scalar(
                out=mv, in0=st_col[:, :, ch0 : ch0 + HCH], scalar1=1.0 / D,
                scalar2=None, op0=OP.mult,
            )
            musq = small.tile([128, HCH], F32, tag="musq")
            nc.vector.tensor_tensor(out=musq, in0=mv[:, 0, :], in1=mv[:, 0, :], op=OP.mult)
            var = small.tile([128, HCH], F32, tag="var")
            nc.vector.tensor_tensor(out=var, in0=mv[:, 1, :], in1=musq, op=OP.subtract)
            _rsqrt_dve(nc, small, var, ru[:, 0, ch0 : ch0 + HCH], HCH, magic_sb)
            nc.vector.tensor_tensor(
                out=ru[:, 1, ch0 : ch0 + HCH], in0=mv[:, 0, :],
                in1=ru[:, 0, ch0 : ch0 + HCH], op=OP.mult,
            )
            # transpose rho / u columns back to rows for this half
            for q, dst in ((0, r_row), (1, u_row)):
                rT = ps_big.tile([1, QH], F16, tag="big", name=f"rT{q}_{th}")
                for ch in range(ch0, ch0 + HCH):
                    nc.tensor.transpose(
                        rT[0:1, (ch - ch0) * 128 : (ch - ch0) * 128 + 128],
                        ru[:, q, ch : ch + 1],
                        identh[:],
                    )
                nc.vector.tensor_copy(out=dst[0:1, th * QH : th * QH + QH], in_=rT[0:1, :])
            pb = ps_big.tile([128, QH], F32, tag="big")
            for j in range(2):
                nc.tensor.matmul(
                    pb[:, j * 512 : j * 512 + 512], (ones_row[0:1, 0:128]),
                    (r_row[0:1, th * QH + j * 512 : th * QH + j * 512 + 512]),
                    start=True, stop=True,
                )
            nc.scalar.activation(
                out=robc[:, th * QH : th * QH + QH], in_=pb[:], func=AF.Copy
            )
            for dt in range(2):
                nc.vector.tensor_tensor(
                    out=xs_sb[:, dt, th * QH : th * QH + QH],
                    in0=xt_sb[:, dt, th * QH : th * QH + QH],
                    in1=robc[:, th * QH : th * QH + QH], op=OP.mult,
                )
        # ---------------- projections ----------------
        def qk_proj(dst, w_sb, bias_i, c_sb, g, ch, use_act):
            ps = ps_big.tile([128, QH], F32, tag="big")
            for j in range(2):
                t0, t1 = ch * QH + j * 512, ch * QH + j * 512 + 512
                for dt in range(2):
                    nc.tensor.matmul(
                        ps[:, j * 512 : j * 512 + 512],
                        (w_sb[:, dt, g * 128 : g * 128 + 128]),
                        (xs_sb[:, dt, t0:t1]),
                        start=(dt == 0), stop=False,
                    )
                nc.tensor.matmul(
                    ps[:, j * 512 : j * 512 + 512],
                    (c_sb[0:1, g * 128 : g * 128 + 128]),
                    (u_row[0:1, t0:t1]),
                    start=False, stop=True,
                )
            dstap = dst[:, g, 0, ch * QH : ch * QH + QH]
            if use_act:
                nc.scalar.activation(
                    out=dstap, in_=ps[:], func=AF.Identity,
                    bias=blob_sb[:, g, bias_i : bias_i + 1],
                )
            else:
                nc.vector.tensor_scalar(
                    out=dstap, in0=ps[:],
                    scalar1=blob_sb[:, g, bias_i : bias_i + 1],
                    scalar2=None, op0=OP.add,
                )

        for g in range(2):
            qk_proj(qt8, wq_sb, 0, cq_sb, g, 0, use_act=(g == 0))
        for g in range(2):
            for ch in range(2):
                qk_proj(kt8, wk_sb, 1, ck_sb, g, ch, use_act=((g + ch) % 2 == 0))

        def v_proj(st, use_act):
            ps = ps_sm.tile([128, D], F32, tag="sm")
            for dt in range(2):
                nc.tensor.matmul(
                    ps[:],
                    (xs_sb[:, dt, st * 128 : st * 128 + 128]),
                    (wv_sb[:, dt, :]),
                    start=(dt == 0), stop=False,
                )
            nc.tensor.matmul(
                ps[:], (u_row[0:1, st * 128 : st * 128 + 128]), (cv_sb[0:1, :]),
                start=False, stop=False,
            )
            nc.tensor.matmul(
                ps[:], (ones_row[0:1, st * 128 : st * 128 + 128]), (bv_sb[0:1, :]),
                start=False, stop=True,
            )
            dst = vaug[:, st, :, 0:HD]
            src = ps.rearrange("p (h f) -> p h f", h=H)
            if use_act:
                nc.scalar.activation(out=dst, in_=src, func=AF.Copy)
            else:
                nc.vector.tensor_copy(out=dst, in_=src)

        for st in range(DT):
            v_proj(st, use_act=(st % 2 == 0))

    # ---------------- attention ----------------
    def flavor(kt):
        return KT_FLAVOR[kt]

    with tc.tile_pool(name="expt", bufs=1) as expt_pool:
        ets = {}    # (h, kt) -> f16 AP [128, QH]

        def qk_step(h, kt):
            g, base = h // 4, 32 * (h % 4)
            fl = flavor(kt)
            ps = ps_big.tile([128, QH], F32, tag="big")
            for qb in range(2):
                nc.tensor.matmul(
                    ps[:, qb * 512 : qb * 512 + 512],
                    (kt8[base : base + 32, g, :, kt * 128 : kt * 128 + 128]),
                    (qt8[base : base + 32, g, :, qb * 512 : qb * 512 + 512]),
                    start=True, stop=(fl != "c"),
                    perf_mode=PM.DoubleRow,
                    tile_position=(base, 0),
                )
            if fl == "c":
                for qb in range(2):
                    nc.tensor.matmul(
                        ps[:, qb * 512 : qb * 512 + 512],
                        (identh[:]),
                        (logm_sb[:, C_IDX[kt], qb * 512 : qb * 512 + 512]),
                        start=False, stop=True,
                    )
            if fl == "b":
                exi = expt_pool.tile(
                    [128, QH], I16, tag="eb", bufs=2 * NB + 2, name=f"eb{h}_{kt}"
                )
                nc.vector.scalar_tensor_tensor(
                    out=exi, in0=ps[:], scalar=ASCH,
                    in1=cmask_sb[:, B_IDX[kt], :], op0=OP.mult, op1=OP.add,
                )
                ets[(h, kt)] = exi.bitcast(F16)
            elif fl == "c":
                ea = expt_pool.tile(
                    [128, QH], F16, tag="ea2", bufs=2 * (NA2 + NA + NC_F) + 2,
                    name=f"ec{h}_{kt}"
                )
                nc.scalar.activation(out=ea, in_=ps[:], func=AF.Exp, bias=shift_sb)
                ets[(h, kt)] = ea[:]
            elif fl == "a2":
                ea = expt_pool.tile(
                    [128, QH], F16, tag="ea2", bufs=2 * (NA2 + NA + NC_F) + 2, name=f"ea2_{h}_{kt}"
                )
                nc.scalar.activation(out=ea, in_=ps[:], func=AF.Exp, bias=shift_sb)
                nc.gpsimd.tensor_tensor(
                    out=ea, in0=ea, in1=expa_sb[:, A_IDX[kt], :], op=OP.mult
                )
                ets[(h, kt)] = ea[:]
            else:
                ea = expt_pool.tile(
                    [128, QH], F16, tag="ea2", bufs=2 * (NA2 + NA + NC_F) + 2,
                    name=f"ea{h}_{kt}"
                )
                nc.scalar.activation(out=ea, in_=ps[:], func=AF.Exp, bias=shift_sb)
                nc.vector.tensor_tensor(
                    out=ea, in0=ea, in1=expa_sb[:, A_IDX[kt], :], op=OP.mult
                )
                ets[(h, kt)] = ea[:]

        pv_tiles = {}

        def pv_part(h, qtg, qi):
            # one query-tile's 16-kt accumulation chain (spread across steps)
            if (h, qtg) not in pv_tiles:
                pv_tiles[(h, qtg)] = ps_sm.tile(
                    [128, 4, HD + 1], F32, tag="sm", name=f"pv{h}_{qtg}"
                )
            pv = pv_tiles[(h, qtg)]
            qt = qtg * 4 + qi
            for kt in range(DT):
                nc.tensor.matmul(
                    pv[:, qi, :],
                    ets[(h, kt)][:, qt * 128 : qt * 128 + 128],
                    vaug[:, kt, h, :],
                    start=(qi == 0 and kt == 0),
                    stop=(qi == 3 and kt == DT - 1),
                )

        def pv_drain(h, qtg):
            pv = pv_tiles.pop((h, qtg))
            dn = small.tile([128, 4], F32, tag="dn")
            nc.vector.reciprocal(out=dn, in_=pv[:, :, HD])
            nc.vector.scalar_tensor_tensor(
                out=ctx_sb[:, qtg * 4 : qtg * 4 + 4, h * HD : h * HD + HD],
                in0=pv[:, :, 0:HD], scalar=1.0,
                in1=dn.unsqueeze(2).broadcast_to([128, 4, HD]),
                op0=OP.mult, op1=OP.mult,
            )
            if qtg == 1:
                for kt in range(DT):
                    del ets[(h, kt)]

        def pv_group(h, qtg):
            for qi in range(4):
                pv_part(h, qtg, qi)
            pv_drain(h, qtg)

        pv7 = {}
        for h in range(H):
            for kt in range(DT):
                qk_step(h, kt)
                if h > 0 and 4 <= kt <= 7:
                    pv_part(h - 1, 0, kt - 4)
                if h > 0 and kt == 8:
                    pv_drain(h - 1, 0)
                if h > 0 and 10 <= kt <= 13:
                    pv_part(h - 1, 1, kt - 10)
                if h > 0 and kt == 14:
                    pv_drain(h - 1, 1)
                if h == H - 1:
                    # kt-major PV for the last head: accumulate as ets arrive
                    if "t" not in pv7:
                        pv7["t"] = ps_sm.tile(
                            [128, 8, HD + 1], F32, tag="sm", name="pv7"
                        )
                    for qt in range(8):
                        nc.tensor.matmul(
                            pv7["t"][:, qt, :],
                            ets[(h, kt)][:, qt * 128 : qt * 128 + 128],
                            vaug[:, kt, h, :],
                            start=(qt == 0 and kt == 0),
                            stop=(qt == 7 and kt == DT - 1),
                        )

        def pv7_drain(qtg):
            pv = pv7["t"]
            dn = small.tile([128, 4], F32, tag="dn")
            nc.vector.reciprocal(out=dn, in_=pv[:, qtg * 4 : qtg * 4 + 4, HD])
            nc.vector.scalar_tensor_tensor(
                out=ctx_sb[:, qtg * 4 : qtg * 4 + 4, (H - 1) * HD : H * HD],
                in0=pv[:, qtg * 4 : qtg * 4 + 4, 0:HD], scalar=1.0,
                in1=dn.unsqueeze(2).broadcast_to([128, 4, HD]),
                op0=OP.mult, op1=OP.mult,
            )
            if qtg == 1:
                for kt in range(DT):
                    if (H - 1, kt) in ets:
                        del ets[(H - 1, kt)]
        pv7_drain(0)

        # -------- post-attention per 512-token chunk --------
        def post_a(qb):
            t0 = qb * 512
            for dt in range(2):
                pt = ps_sm.tile([128, 512], F16, tag="sm")
                for qq in range(4):
                    qt = qb * 4 + qq
                    nc.tensor.transpose(
                        pt[:, qq * 128 : qq * 128 + 128],
                        ctx_sb[:, qt, dt * 128 : dt * 128 + 128],
                        identh[:],
                    )
                if dt == 0:
                    nc.scalar.activation(
                        out=ctxT[:, dt, t0 : t0 + 512], in_=pt[:], func=AF.Copy
                    )
                else:
                    nc.vector.tensor_copy(out=ctxT[:, dt, t0 : t0 + 512], in_=pt[:])
            for mt in range(2):
                ps = ps_big.tile([128, QH], F32, tag="big")
                for dt in range(2):
                    nc.tensor.matmul(
                        ps[:, 0:512],
                        (wo_sb[:, dt, mt * 128 : mt * 128 + 128]),
                        (ctxT[:, dt, t0 : t0 + 512]),
                        start=(dt == 0), stop=(dt == 1),
                    )
                nc.vector.scalar_tensor_tensor(
                    out=yT[:, mt, t0 : t0 + 512], in0=ps[:, 0:512],
                    scalar=blob_sb[:, mt, 2:3], in1=xt_q[:, mt, t0 : t0 + 512],
                    op0=OP.add, op1=OP.add,
                )
            ysq = small.tile([128, 2, 512], F16, tag="ysq")
            for dt in range(2):
                nc.vector.tensor_tensor(
                    out=ysq[:, dt, :], in0=yT[:, dt, t0 : t0 + 512],
                    in1=yT[:, dt, t0 : t0 + 512], op=OP.mult,
                )
            st2 = ps_sm.tile([128, 2, 4], F32, tag="sm")
            for q in range(2):
                for ch in range(4):
                    for dt in range(2):
                        src = (
                            yT[:, dt, t0 + ch * 128 : t0 + ch * 128 + 128]
                            if q == 0
                            else ysq[:, dt, ch * 128 : ch * 128 + 128]
                        )
                        nc.tensor.matmul(
                            st2[:, q, ch : ch + 1], (src), (ones_col[:]),
                            start=(q == 0 and ch == 0 and dt == 0),
                            stop=(q == 1 and ch == 3 and dt == 1),
                        )
            mv2 = small.tile([128, 2, 4], F32, tag="mv2")
            nc.vector.tensor_scalar(
                out=mv2, in0=st2, scalar1=1.0 / D, scalar2=None, op0=OP.mult
            )
            ms2 = small.tile([128, 4], F32, tag="ms2")
            nc.vector.tensor_tensor(out=ms2, in0=mv2[:, 0, :], in1=mv2[:, 0, :], op=OP.mult)
            var2 = small.tile([128, 4], F32, tag="var2")
            nc.vector.tensor_tensor(out=var2, in0=mv2[:, 1, :], in1=ms2, op=OP.subtract)
            ru2 = small.tile([128, 2, 4], F16, tag="ru2")
            _rsqrt_dve(nc, small, var2, ru2[:, 0, :], 4, magic_sb)
            nc.vector.tensor_tensor(
                out=ru2[:, 1, :], in0=mv2[:, 0, :], in1=ru2[:, 0, :], op=OP.mult
            )
            rows2 = ps_sm.tile([1, 2, 512], F16, tag="sm")
            for q in range(2):
                for ch in range(4):
                    nc.tensor.transpose(
                        rows2[0:1, q, ch * 128 : ch * 128 + 128],
                        ru2[:, q, ch : ch + 1],
                        identh[:],
                    )
            r2r = small.tile([1, 512], F16, tag="r2r")
            nc.vector.tensor_copy(out=r2r, in_=rows2[0:1, 0, :])
            nc.vector.tensor_copy(out=u2_row[0:1, t0 : t0 + 512], in_=rows2[0:1, 1, :])
            pb = ps_big.tile([128, QH], F32, tag="big")
            nc.tensor.matmul(
                pb[:, 0:512], (ones_row[0:1, 0:128]), (r2r[:]), start=True, stop=True
            )
            nc.tensor.matmul(
                pb[:, 512:1024], (ones_row[0:1, 0:128]),
                (u2_row[0:1, t0 : t0 + 512]), start=True, stop=True,
            )
            nc.scalar.activation(
                out=ro2bc[:, t0 : t0 + 512], in_=pb[:, 0:512], func=AF.Copy
            )
            u2bc = small.tile([128, 512], F16, tag="u2bc")
            nc.vector.tensor_copy(out=u2bc, in_=pb[:, 512:1024])
            for dt in range(2):
                nc.vector.tensor_tensor(
                    out=y2[:, dt, t0 : t0 + 512], in0=yT[:, dt, t0 : t0 + 512],
                    in1=ro2bc[:, t0 : t0 + 512], op=OP.mult,
                )
                nc.vector.tensor_tensor(
                    out=y2[:, dt, t0 : t0 + 512], in0=y2[:, dt, t0 : t0 + 512],
                    in1=u2bc, op=OP.subtract,
                )
        def post_b(qb):
            t0 = qb * 512
            with tc.tile_pool(name=f"mlp{qb}", bufs=1) as mlp_pool:
                hT = mlp_pool.tile([128, 8, 512], F16, tag="hT")
                for mt in range(8):
                    ps = ps_big.tile([128, QH], F32, tag="big")
                    for dt in range(2):
                        nc.tensor.matmul(
                            ps[:, 0:512],
                            (w1_sb[:, dt, mt * 128 : mt * 128 + 128]),
                            (y2[:, dt, t0 : t0 + 512]),
                            start=(dt == 0), stop=(dt == 1),
                        )
                    nc.scalar.activation(
                        out=hT[:, mt, :], in_=ps[:, 0:512],
                        func=AF.Gelu, bias=b1_sb[:, mt, :],
                    )
                ps2 = [ps_big.tile([128, QH], F32, tag="big", name=f"m2_{qb}_{m}")
                       for m in range(2)]
                for j in range(8):
                    for m in range(2):
                        nc.tensor.matmul(
                            ps2[m][:, 0:512],
                            (w2_sb[:, j, m * 128 : m * 128 + 128]),
                            (hT[:, j, :]),
                            start=(j == 0), stop=(j == 7),
                        )
                for m in range(2):
                    nc.vector.scalar_tensor_tensor(
                        out=outT[:, m, t0 : t0 + 512], in0=ps2[m][:, 0:512],
                        scalar=blob_sb[:, m, 3:4], in1=yT[:, m, t0 : t0 + 512],
                        op0=OP.add, op1=OP.add,
                    )
            for ch in range(4):
                po = ps_sm.tile([128, D], F16, tag="sm")
                for mt in range(2):
                    nc.tensor.transpose(
                        po[:, mt * 128 : mt * 128 + 128],
                        outT[:, mt, t0 + ch * 128 : t0 + ch * 128 + 128],
                        identh[:],
                    )
                gi = qb * 4 + ch
                if ch % 2 == 0:
                    nc.scalar.activation(out=out_sb[:, gi, :], in_=po[:], func=AF.Copy)
                else:
                    nc.vector.tensor_copy(out=out_sb[:, gi, :], in_=po[:])
                nc.sync.dma_start(
                    out=out_d.rearrange("(g p) d -> p g d", p=128)[:, gi, :],
                    in_=out_sb[:, gi, :],
                )

        post_a(0)
        pv7_drain(1)
        post_a(1)
        post_b(0)
        post_b(1)


_NC_CACHE = {}


def _get_nc():
    if "nc" not in _NC_CACHE:
        nc = bacc.Bacc("TRN2", target_bir_lowering=False, debug=False)
        with tile.TileContext(nc) as tc:
            with ExitStack() as ctx:
                _emit(ctx, tc)
        nc.compile()
        _NC_CACHE["nc"] = nc
    return _NC_CACHE["nc"]


def _prep_common(inputs):
    f = lambda k: np.asarray(inputs[k], np.float32)
    g1, b1n = f("ln1_g"), f("ln1_b")
    g2, b2n = f("ln2_g"), f("ln2_b")
    wq, wk, wv, wo = f("wq"), f("wk"), f("wv"), f("wo")
    w1, w2 = f("w1"), f("w2")

    wq_eff = (wq * g1[None, :]).T * SCALE
    wk_eff = (wk * g1[None, :]).T
    wv_eff = (wv * g1[None, :]).T
    w1_eff = (w1 * g2[None, :]).T
    bias_q = (f("bq") + wq @ b1n) * SCALE
    bias_k = f("bk") + wk @ b1n
    bias_v = f("bv") + wv @ b1n
    bias_1 = f("b1") + w1 @ b2n
    blob = np.stack([bias_q, bias_k, f("bo"), f("b2")], axis=1)
    crow = np.stack(
        [-wq_eff.sum(0), -wk_eff.sum(0), -wv_eff.sum(0), bias_v], axis=0
    )
    return {
        "wq": np.ascontiguousarray(wq_eff).astype(np.float16),
        "wk": np.ascontiguousarray(wk_eff).astype(np.float16),
        "wv": np.ascontiguousarray(wv_eff).astype(np.float16),
        "wo": np.ascontiguousarray(wo.T).astype(np.float16),
        "w1": np.ascontiguousarray(w1_eff).astype(np.float16),
        "w2": np.ascontiguousarray(w2.T).astype(np.float16),
        "blob": np.ascontiguousarray(blob).astype(np.float32),
        "b1": bias_1.reshape(4 * D, 1).astype(np.float32),
        "crow": np.ascontiguousarray(crow).astype(np.float16),
        "onesrow": np.ones((1, S), np.float16),
    }


def _run(inputs, trace=False):
    x = np.asarray(inputs["x"], np.float32)
    adj = np.asarray(inputs["adj_mask"]).astype(bool)
    ea = np.asarray(inputs["edge_attr"], np.float32).reshape(-1)
    ei = np.asarray(inputs["edge_index"]).astype(np.int64)

    bias2d = np.zeros((S, S), np.float32)
    bias2d[ei[0], ei[1]] = np.clip(ea, -5.0, 5.0)

    common = _prep_common(inputs)
    in_maps = []
    for c in range(NCORES):
        b, qh = c // 2, c % 2
        ordr = np.arange(S)
        if qh == 1:
            ordr = np.concatenate([ordr[QH:], ordr[:QH]])
        xc = x[b][ordr]
        adj_kq = adj[b][ordr[:QH]][:, ordr].T      # [k rotated, q(1024)]
        bias_kq = bias2d[ordr[:QH]][:, ordr].T
        expa = np.where(adj_kq, np.exp(bias_kq), 0.0).astype(np.float16)
        cm = np.where(
            adj_kq, ASCH * (bias_kq + SHIFT) + BSCH, -60000.0
        ).astype(np.float16)
        lg = np.where(adj_kq, bias_kq, -30.0).astype(np.float16)
        er = expa.reshape(DT, 128, QH)
        cr = cm.reshape(DT, 128, QH)
        lr = lg.reshape(DT, 128, QH)
        in_maps.append(
            {
                "xt": np.ascontiguousarray(xc.T).astype(np.float16),
                "expa": np.ascontiguousarray(
                    er[AKTS].reshape(KA * 128, QH)
                ),
                "cmask": np.ascontiguousarray(
                    cr[BKTS].reshape(KB * 128, QH)
                ),
                "logm": np.ascontiguousarray(
                    lr[CKTS].reshape(KC * 128, QH)
                ),
                **common,
            }
        )

    nc = _get_nc()
    res = run_bass_kernel_spmd(nc, in_maps, core_ids=list(range(NCORES)), trace=trace)
    outs = [res.results[c]["out"].astype(np.float32) for c in range(NCORES)]
    y = np.stack(
        [np.concatenate([outs[2 * b], outs[2 * b + 1]], axis=0) for b in range(B)],
        axis=0,
    )
    return y, res


def kernel(**inputs) -> np.ndarray:
    y, _ = _run(inputs, trace=False)
    return y
